# revision 1
# baseline (speedup 1.0000x reference)
"""ODE-RNN Trainium2 kernel.

Math (matches jax reference):
  per step t (times from batch[0,:,0], shared across batch):
    hp = ODE-integrate dh/dt = tanh(h @ A)  over [t_prev, t]  (A = W1.T @ W2.T,
         biases are zero), Heun-RK3: k1=f(h), k2=f(h+dt/3 k1), k3=f(h+2dt/3 k2),
         hp = h + dt/4 (k1 + 3 k3)
    gru: r = sig(gi_r + gh_r), z = sig(gi_z + gh_z), n = tanh(gi_n + r*gh_n)
    h = hp + (m*(1-z))*(n - hp)

Device layout: transposed (H on partitions, batch on free), batch sharded 8
ways (32 rows/core), weights replicated. All matmuls run in fp16 (fp32 PSUM
accumulate; ~3e-4 final absmax-rel, validated against dopri5 on host) — fp16
halves weight-load time (FWL) and single-passes the PE vs fp32's hi/lo split.

Critical path: state carried as (hp, e) with h = hp + e; per-step scaled A
copies ((dt/3)A, (2dt/3)A in fp16) are precomputed on host and DMA-streamed,
so stage combines are pure PSUM accumulation; gh = W_hh@h + W_hh@((3dt/4)tt)
splits the GRU matmul into an early off-path part and a small tail.
"""
import numpy as np

import concourse.bass as bass
import concourse.bacc as bacc
import concourse.tile as tile
from concourse import mybir
from concourse.bass_utils import run_bass_kernel_spmd

B, T, H, D = 256, 64, 256, 512
NCORES = 8
BL = B // NCORES          # 32 batch rows per core
KT = H // 128             # 2 contraction tiles
F32 = mybir.dt.float32
F16 = mybir.dt.float16
AF = mybir.ActivationFunctionType
OP = mybir.AluOpType

STAGES = 3    # 3 = Heun-RK3 (~3.1e-4), 2 = midpoint (~5.2e-4, one stage cheaper)


def _build_program(dts, repeat=1):
    nc = bacc.Bacc(None, target_bir_lowering=False)

    a_d = nc.dram_tensor("a16", [128, KT * H], F16, kind="ExternalInput")
    whh_d = nc.dram_tensor("whh16", [128, KT, 3 * H], F16, kind="ExternalInput")
    a1_d = nc.dram_tensor("a1s", [128, T, KT * H], F16, kind="ExternalInput")
    a1x2_d = None
    if STAGES == 3:
        a1x2_d = nc.dram_tensor("a1x2s", [128, T, KT * H], F16,
                                kind="ExternalInput")
    foldw_d = nc.dram_tensor("foldw", [96, 128], F16, kind="ExternalInput")
    foldx_d = nc.dram_tensor("foldx", [96, T, 2 * BL], F16, kind="ExternalInput")
    mrow_d = nc.dram_tensor("mrow", [1, T * BL], F32, kind="ExternalInput")
    gi_d = nc.dram_tensor("gi_n", [T, 128, KT, BL], F32, kind="ExternalInput")
    out_d = nc.dram_tensor("h_out", [KT, 128, BL], F32, kind="ExternalOutput")

    with tile.TileContext(nc) as tc:
        with (
            tc.tile_pool(name="const", bufs=1) as const,
            tc.tile_pool(name="state", bufs=2) as state,
            tc.tile_pool(name="tmp", bufs=3) as tmp,
            tc.tile_pool(name="ps_stage", bufs=2, space="PSUM") as ps_stage,
            tc.tile_pool(name="ps_r", bufs=2, space="PSUM") as ps_r,
            tc.tile_pool(name="ps_z", bufs=2, space="PSUM") as ps_z,
            tc.tile_pool(name="ps_n", bufs=2, space="PSUM") as ps_n,
        ):
            # ---- preload constants ----
            a_sb = const.tile([128, KT * H], F16)
            nc.sync.dma_start(out=a_sb, in_=a_d[:, :])
            a1_sb = const.tile([128, T, KT * H], F16)
            for t0 in range(0, T, 8):      # chunked: stay under 64KB/partition/desc
                nc.sync.dma_start(out=a1_sb[:, t0:t0 + 8, :],
                                  in_=a1_d[:, t0:t0 + 8, :])
            a1x2_sb = None
            if STAGES == 3:
                a1x2_sb = const.tile([128, T, KT * H], F16)
                for t0 in range(0, T, 8):
                    nc.sync.dma_start(out=a1x2_sb[:, t0:t0 + 8, :],
                                      in_=a1x2_d[:, t0:t0 + 8, :])
            whh_sb = const.tile([128, KT, 3 * H], F16)
            nc.sync.dma_start(out=whh_sb, in_=whh_d[:, :, :])
            foldw_sb = const.tile([96, 128], F16)
            nc.sync.dma_start(out=foldw_sb, in_=foldw_d[:, :])
            foldx_sb = const.tile([96, T, 2 * BL], F16)
            nc.sync.dma_start(out=foldx_sb, in_=foldx_d[:, :, :])
            m_sb = const.tile([128, T * BL], F32)
            mrow_ap = mrow_d[0, :]
            nc.sync.dma_start(
                out=m_sb,
                in_=bass.AP(tensor=mrow_ap.tensor, offset=mrow_ap.offset,
                            ap=[[0, 128], [1, T * BL]]),
            )
            gi_sb = const.tile([128, T, KT, BL], F32)
            for t in range(T):
                nc.sync.dma_start(out=gi_sb[:, t, :, :], in_=gi_d[t, :, :, :])

            def lhsT_of(sb, k, m):
                return sb[:, k * H + m * 128:k * H + (m + 1) * 128]

            def whh_lhsT(k, g):
                return whh_sb[:, k, g * 128:(g + 1) * 128]

            def body():
                hp0 = state.tile([128, KT, BL], F32, tag="hp")
                nc.vector.memset(hp0, 0.0)
                hp016 = state.tile([128, KT, BL], F16, tag="hp16")
                nc.vector.memset(hp016, 0.0)
                e0 = state.tile([128, KT, BL], F32, tag="e")
                nc.vector.memset(e0, 0.0)
                e016 = state.tile([128, KT, BL], F16, tag="e16")
                nc.vector.memset(e016, 0.0)

                for t in range(T):
                    dt = float(dts[t])
                    a1t = a1_sb[:, t, :]

                    # h(t) = hp + e, fp32 + fp16 copy (off critical path)
                    hcur = tmp.tile([128, KT, BL], F32, tag="hcur")
                    nc.vector.tensor_add(hcur, hp0, e0)
                    hcur16 = tmp.tile([128, KT, BL], F16, tag="hcur16")
                    nc.vector.tensor_copy(hcur16, hcur)

                    # GRU gate psums: exact block-diag fold MMs (one per bank)
                    psr_t = ps_r.tile([128, 2, BL], F32, tag="psr")
                    psz_t = ps_z.tile([128, 2, BL], F32, tag="psz")
                    psn_t = ps_n.tile([128, 2, BL], F32, tag="psn")
                    nc.tensor.matmul(psr_t[:, :, :], foldw_sb[0:10, :],
                                     foldx_sb[0:10, t, :],
                                     start=True, stop=False, skip_group_check=True)
                    nc.tensor.matmul(psz_t[:, :, :], foldw_sb[32:42, :],
                                     foldx_sb[32:42, t, :],
                                     start=True, stop=False, skip_group_check=True)
                    nc.tensor.matmul(psn_t[:, :, :], foldw_sb[64:68, :],
                                     foldx_sb[64:68, t, :],
                                     start=True, stop=False, skip_group_check=True)
                    psg = [psr_t, psz_t, psn_t]

                    # GRU main part: gh += W_hh @ h  (off critical path)
                    for g in range(6):
                        for k in range(KT):
                            nc.tensor.matmul(psg[g // 2][:, g % 2, :], whh_lhsT(k, g),
                                             hcur16[:, k, :], start=False, stop=False,
                                             skip_group_check=True)

                    # ---- ODE stage 1: U = A.T hp0 + A.T e0  (bank alpha) ----
                    ps1 = ps_stage.tile([128, 2, BL], F32, tag="stage")
                    for m in range(2):
                        for k in range(KT):
                            nc.tensor.matmul(ps1[:, m, :], lhsT_of(a_sb, k, m),
                                             hp016[:, k, :],
                                             start=(m == 0 and k == 0), stop=False,
                                             skip_group_check=True)
                    for m in range(2):
                        for k in range(KT):
                            nc.tensor.matmul(ps1[:, m, :], lhsT_of(a_sb, k, m),
                                             e016[:, k, :], start=False, stop=False,
                                             skip_group_check=True)
                    k1h = tmp.tile([128, KT, BL], F16, tag="k1h")
                    nc.scalar.activation(k1h, ps1, AF.Tanh)

                    # ---- stage 2 (in place): pre2 = U + (c2 dt A).T k1 ----
                    for m in range(2):
                        for k in range(KT):
                            nc.tensor.matmul(ps1[:, m, :], lhsT_of(a1t, k, m),
                                             k1h[:, k, :], start=False,
                                             stop=(m == 1 and k == KT - 1),
                                             skip_group_check=True)
                    k2h = tmp.tile([128, KT, BL], F16, tag="k2h")
                    nc.scalar.activation(k2h, ps1, AF.Tanh)

                    if STAGES == 3:
                        # ---- stage 3 (bank beta): pre3 = A.T h + (2dt/3 A).T k2 ----
                        a1x2t = a1x2_sb[:, t, :]
                        ps3 = ps_stage.tile([128, 2, BL], F32, tag="stage")
                        for m in range(2):
                            for k in range(KT):
                                nc.tensor.matmul(ps3[:, m, :], lhsT_of(a_sb, k, m),
                                                 hcur16[:, k, :],
                                                 start=(m == 0 and k == 0),
                                                 stop=False, skip_group_check=True)
                        for m in range(2):
                            for k in range(KT):
                                nc.tensor.matmul(ps3[:, m, :], lhsT_of(a1x2t, k, m),
                                                 k2h[:, k, :], start=False,
                                                 stop=(m == 1 and k == KT - 1),
                                                 skip_group_check=True)
                        k3h = tmp.tile([128, KT, BL], F16, tag="k3h")
                        nc.scalar.activation(k3h, ps3, AF.Tanh)

                        # tt = k1/3 + k3 (fp16); hp = h + (3dt/4) tt
                        tt16 = tmp.tile([128, KT, BL], F16, tag="tt16")
                        nc.vector.scalar_tensor_tensor(tt16, k1h, 1.0 / 3.0, k3h,
                                                       op0=OP.mult, op1=OP.add)
                        hscale = 3 * dt / 4
                    else:
                        # midpoint: hp = h + dt k2
                        tt16 = k2h
                        hscale = dt
                    hp = state.tile([128, KT, BL], F32, tag="hp")
                    nc.vector.scalar_tensor_tensor(hp, tt16, hscale, hcur,
                                                   op0=OP.mult, op1=OP.add)
                    hp16 = state.tile([128, KT, BL], F16, tag="hp16")
                    nc.vector.tensor_copy(hp16, hp)
                    # tts = hscale * tt (fp16) for the GRU tail
                    tts = tmp.tile([128, KT, BL], F16, tag="tts")
                    nc.vector.tensor_scalar_mul(tts, tt16, hscale)

                    # ---- GRU tail: gh += W_hh @ tts; r gates first ----
                    for g in (0, 1, 4, 5, 2, 3):
                        for k in range(KT):
                            nc.tensor.matmul(psg[g // 2][:, g % 2, :], whh_lhsT(k, g),
                                             tts[:, k, :], start=False,
                                             stop=(g in (2, 3) and k == KT - 1),
                                             skip_group_check=True)

                    r = tmp.tile([128, KT, BL], F32, tag="r")
                    nc.scalar.activation(r, psg[0], AF.Sigmoid)
                    tmpn = tmp.tile([128, KT, BL], F32, tag="tmpn")
                    nc.vector.tensor_mul(tmpn, psg[2], r)
                    argn = tmp.tile([128, KT, BL], F32, tag="argn")
                    nc.vector.tensor_add(argn, tmpn, gi_sb[:, t, :, :])

                    zc = tmp.tile([128, KT, BL], F32, tag="zc")
                    nc.scalar.activation(zc, psg[1], AF.Sigmoid, scale=-1.0)
                    m_slice = m_sb[:, t * BL:(t + 1) * BL]
                    m_ap = bass.AP(tensor=m_slice.tensor, offset=m_slice.offset,
                                   ap=[list(m_slice.ap[0]), [0, KT], [1, BL]])
                    w = tmp.tile([128, KT, BL], F32, tag="w")
                    nc.gpsimd.tensor_mul(w, zc, m_ap)

                    n = tmp.tile([128, KT, BL], F32, tag="n")
                    nc.scalar.activation(n, argn, AF.Tanh)

                    d = tmp.tile([128, KT, BL], F32, tag="d")
                    nc.vector.tensor_sub(d, n, hp)
                    e16n = state.tile([128, KT, BL], F16, tag="e16")
                    nc.vector.tensor_mul(e16n, w, d)    # fp16, feeds next U wave
                    e = state.tile([128, KT, BL], F32, tag="e")
                    nc.vector.tensor_mul(e, w, d)       # fp32 state (off chain)

                    hp0, hp016, e0, e016 = hp, hp16, e, e16n

                hfin = tmp.tile([128, KT, BL], F32, tag="hcur")
                nc.vector.tensor_add(hfin, hp0, e0)
                return hfin

            if repeat == 1:
                hfin = body()
            else:
                with tc.For_i(0, repeat, 1):
                    hfin = body()

            for k in range(KT):
                nc.sync.dma_start(out=out_d[k, :, :], in_=hfin[:, k, :])

    nc.finalize()
    return nc


def _prepare_inputs(batch, mask, W1, b1, W2, b2, W_ih, b_ih, W_hh, b_hh):
    batch = np.asarray(batch, np.float32)
    mask = np.asarray(mask, np.float32)
    W1 = np.asarray(W1, np.float32); b1 = np.asarray(b1, np.float32)
    W2 = np.asarray(W2, np.float32); b2 = np.asarray(b2, np.float32)
    W_ih = np.asarray(W_ih, np.float32); b_ih = np.asarray(b_ih, np.float32)
    W_hh = np.asarray(W_hh, np.float32); b_hh = np.asarray(b_hh, np.float32)

    A = (W1.T.astype(np.float64) @ W2.T.astype(np.float64)).astype(np.float32)
    c = (b1.astype(np.float64) @ W2.T.astype(np.float64) + b2).astype(np.float32)
    assert np.abs(c).max() == 0.0, "nonzero ODE bias not wired into ACT bias"

    times = batch[0, :, 0].astype(np.float64)
    dts = np.diff(np.concatenate([[0.0], times]))

    def a_blocks(M, dtype=np.float16):   # [H, H] -> [128, KT*H] k-tile concat
        return np.ascontiguousarray(np.concatenate(
            [M[k * 128:(k + 1) * 128, :] for k in range(KT)], axis=1)).astype(dtype)

    a16 = a_blocks(A)
    c2 = 1 / 3 if STAGES == 3 else 1 / 2
    a1s = np.ascontiguousarray(np.stack(
        [a_blocks((A.astype(np.float64) * (c2 * d)).astype(np.float32))
         for d in dts]).transpose(1, 0, 2))              # [128,T,KT*H] fp16
    a1x2s = None
    if STAGES == 3:
        a1x2s = np.ascontiguousarray(np.stack(
            [a_blocks((A.astype(np.float64) * (2 * d / 3)).astype(np.float32))
             for d in dts]).transpose(1, 0, 2))
    WhhT = np.ascontiguousarray(W_hh.T)
    whh16 = np.ascontiguousarray(
        np.stack([WhhT[k * 128:(k + 1) * 128, :] for k in range(KT)], axis=1)
    ).astype(np.float16)

    # fold weights: exact fp16 split of W_ih and (b_ih+b_hh) per gate half.
    # lhsT row blocks per region: [Whi, Wlo, Whi, bhi, blo] pairing with rhs
    # rows [xhi, xhi, xlo, 1, 1]; n-gate: [bhi, blo] with ones.
    bsum = b_ih + b_hh
    foldw = np.zeros((96, 128), np.float16)
    for reg in range(4):                                 # r0 r1 z0 z1
        wslice = W_ih[reg * 128:(reg + 1) * 128, 0]
        whi = wslice.astype(np.float16)
        wlo = (wslice - whi.astype(np.float32)).astype(np.float16)
        bshi = bsum[reg * 128:(reg + 1) * 128].astype(np.float16)
        bslo = (bsum[reg * 128:(reg + 1) * 128]
                - bshi.astype(np.float32)).astype(np.float16)
        base = (reg // 2) * 32 + (reg % 2) * 5           # r: 0/5, z: 32/37
        foldw[base + 0] = whi
        foldw[base + 1] = wlo
        foldw[base + 2] = whi
        foldw[base + 3] = bshi
        foldw[base + 4] = bslo
    for reg in range(2):                                 # n0 n1 (b_hh only)
        bn = b_hh[2 * H + reg * 128:2 * H + (reg + 1) * 128]
        bnhi = bn.astype(np.float16)
        bnlo = (bn - bnhi.astype(np.float32)).astype(np.float16)
        foldw[64 + reg * 2 + 0] = bnhi
        foldw[64 + reg * 2 + 1] = bnlo

    xs = batch[:, :, 1]
    gi_n_full = (xs[:, :, None] * W_ih[None, None, 2 * H:, 0]
                 + b_ih[None, None, 2 * H:]).astype(np.float32)  # [B,T,H]

    in_maps = []
    for ci in range(NCORES):
        bs = slice(ci * BL, (ci + 1) * BL)
        xs_c = xs[bs].T                                  # [T, BL]
        xhi = xs_c.astype(np.float16)
        xlo = (xs_c - xhi.astype(np.float32)).astype(np.float16)
        foldx = np.zeros((96, T, 2 * BL), np.float16)
        for reg01, sl in ((0, slice(0, BL)), (1, slice(BL, 2 * BL))):
            for zbase in (0, 32):                        # r rows, z rows (same rhs)
                base = zbase + reg01 * 5
                foldx[base + 0, :, sl] = xhi
                foldx[base + 1, :, sl] = xhi
                foldx[base + 2, :, sl] = xlo
                foldx[base + 3, :, sl] = 1.0
                foldx[base + 4, :, sl] = 1.0
            foldx[64 + reg01 * 2 + 0, :, sl] = 1.0       # n ones
            foldx[64 + reg01 * 2 + 1, :, sl] = 1.0
        mrow = np.ascontiguousarray(mask[bs].T.reshape(1, -1)).astype(np.float32)
        gi_c = gi_n_full[bs].transpose(1, 2, 0)          # [T, H, BL]
        gi_c = np.ascontiguousarray(
            gi_c.reshape(T, KT, 128, BL).transpose(0, 2, 1, 3))
        im = {
            "a16": a16, "whh16": whh16, "a1s": a1s,
            "foldw": foldw, "foldx": np.ascontiguousarray(foldx),
            "mrow": mrow, "gi_n": gi_c,
        }
        if STAGES == 3:
            im["a1x2s"] = a1x2s
        in_maps.append(im)
    return dts, in_maps


def kernel(batch, mask, W1, b1, W2, b2, W_ih, b_ih, W_hh, b_hh):
    dts, in_maps = _prepare_inputs(batch, mask, W1, b1, W2, b2,
                                   W_ih, b_ih, W_hh, b_hh)
    nc = _build_program([float(d) for d in dts])
    res = run_bass_kernel_spmd(nc, in_maps, core_ids=list(range(NCORES)))

    out = np.empty((B, H), np.float32)
    for ci in range(NCORES):
        ho = res.results[ci]["h_out"]                    # [KT, 128, BL]
        for k in range(KT):
            out[ci * BL:(ci + 1) * BL, k * 128:(k + 1) * 128] = ho[k].T
    return out



# revision 24
# speedup vs baseline: 1.6397x; 1.6397x over previous
"""ODE-RNN Trainium2 kernel (midpoint + persistent-U).

Math (matches jax reference; validated 9.1e-4 relmax on host):
  per step t (times from batch[0,:,0], shared across batch):
    hp = h + dt*k2, k1 = tanh(A.T h), k2 = tanh(A.T (h + dt/2 k1))
         (A = W1.T @ W2.T, biases zero; midpoint RK2)
    gru: r = sig(gr), zc = 1-z = sig(-gz), n = tanh(gi_n + r*gh_n)
    w = mask*zc;  h' = hp + w*(n - hp) = hp - w*hp + w*n

Key idea: carry U = A.T@h in a persistent PSUM bank across steps:
    U' = U + dt*(A.T k2) + A.T(w n) - A.T(w hp)
so the per-step stage-1 recompute (8 matmuls off h-state) disappears and
the critical path is 4 ACT ops (tanh k1, tanh k2, sigmoid rz, tanh n) +
3 small matmul groups + 4 DVE ops. z-gate weights are negated on host so
one sigmoid yields [r, 1-z] in a single ACT. Stage-2 pre-activation is
rebuilt fresh each step (ps2 = A.T h16 + (dt/2 A).T k1) which keeps U's
accumulation error bounded (validated on host).

Device layout: transposed (H on partitions, batch on free), batch sharded
8 ways (32 rows/core), weights replicated, all matmuls fp16 with fp32
PSUM accumulate. Per-step scaled-A copies ((dt/2)A, dt*A fp16) are
precomputed on host and preloaded to SBUF.
"""
import numpy as np

import concourse.bass as bass
import concourse.bacc as bacc
import concourse.tile as tile
from concourse import mybir
from concourse.bass_utils import run_bass_kernel_spmd

B, T, H, D = 256, 64, 256, 512
NCORES = 8
BL = B // NCORES          # 32 batch rows per core
KT = H // 128             # 2 contraction tiles
F32 = mybir.dt.float32
F16 = mybir.dt.float16
AF = mybir.ActivationFunctionType
OP = mybir.AluOpType


def _build_program(dts, repeat=1):
    nc = bacc.Bacc(None, target_bir_lowering=False)

    a_d = nc.dram_tensor("a16", [128, KT * H], F16, kind="ExternalInput")
    an_d = nc.dram_tensor("a16n", [128, KT * H], F16, kind="ExternalInput")
    whh_d = nc.dram_tensor("whh16", [128, KT, 3 * H], F16, kind="ExternalInput")
    a1_d = nc.dram_tensor("a1s", [128, T, KT * H], F16, kind="ExternalInput")
    wrs_d = nc.dram_tensor("wrs", [128, T, KT * H], F16, kind="ExternalInput")
    foldw_d = nc.dram_tensor("foldw", [96, 128], F16, kind="ExternalInput")
    foldx_d = nc.dram_tensor("foldx", [96, T, 2 * BL], F16, kind="ExternalInput")
    mrow_d = nc.dram_tensor("mrow", [1, T * BL], F32, kind="ExternalInput")
    gi_d = nc.dram_tensor("gi_n", [T, 128, KT, BL], F32, kind="ExternalInput")
    out_d = nc.dram_tensor("h_out", [KT, 128, BL], F32, kind="ExternalOutput")

    with tile.TileContext(nc) as tc:
        with (
            tc.tile_pool(name="const", bufs=1) as const,
            tc.tile_pool(name="state", bufs=2) as state,
            tc.tile_pool(name="tmp", bufs=3) as tmp,
            tc.tile_pool(name="ps_u", bufs=1, space="PSUM") as ps_u,
            tc.tile_pool(name="ps_2", bufs=1, space="PSUM") as ps_2,
            tc.tile_pool(name="ps_r", bufs=2, space="PSUM") as ps_r,
            tc.tile_pool(name="ps_z", bufs=2, space="PSUM") as ps_z,
            tc.tile_pool(name="ps_n", bufs=2, space="PSUM") as ps_n,
        ):
            # ---- preload constants ----
            a_sb = const.tile([128, KT * H], F16)
            nc.sync.dma_start(out=a_sb, in_=a_d[:, :])
            an_sb = const.tile([128, KT * H], F16)
            nc.sync.dma_start(out=an_sb, in_=an_d[:, :])
            a1_sb = const.tile([128, T, KT * H], F16)
            wrs_sb = const.tile([128, T, KT * H], F16)
            for t0 in range(0, T, 8):      # chunked: stay under 64KB/partition/desc
                nc.sync.dma_start(out=a1_sb[:, t0:t0 + 8, :],
                                  in_=a1_d[:, t0:t0 + 8, :])
                nc.sync.dma_start(out=wrs_sb[:, t0:t0 + 8, :],
                                  in_=wrs_d[:, t0:t0 + 8, :])
            whh_sb = const.tile([128, KT, 3 * H], F16)
            nc.sync.dma_start(out=whh_sb, in_=whh_d[:, :, :])
            foldw_sb = const.tile([96, 128], F16)
            nc.sync.dma_start(out=foldw_sb, in_=foldw_d[:, :])
            foldx_sb = const.tile([96, T, 2 * BL], F16)
            nc.sync.dma_start(out=foldx_sb, in_=foldx_d[:, :, :])
            m_sb = const.tile([128, T * BL], F32)
            mrow_ap = mrow_d[0, :]
            nc.sync.dma_start(
                out=m_sb,
                in_=bass.AP(tensor=mrow_ap.tensor, offset=mrow_ap.offset,
                            ap=[[0, 128], [1, T * BL]]),
            )
            gi_sb = const.tile([128, T, KT, BL], F32)
            for t in range(T):
                nc.sync.dma_start(out=gi_sb[:, t, :, :], in_=gi_d[t, :, :, :])

            def lhsT_of(sb, k, m):
                return sb[:, k * H + m * 128:k * H + (m + 1) * 128]

            def whh_lhsT(k, g):
                return whh_sb[:, k, g * 128:(g + 1) * 128]

            def body():
                h16_0 = state.tile([128, KT, BL], F16, tag="h16")
                nc.vector.memset(h16_0, 0.0)

                # init U = A.T @ h0 (= 0) via matmul start=True
                psU = ps_u.tile([128, 2, BL], F32, tag="U")
                for m in range(2):
                    for k in range(KT):
                        nc.tensor.matmul(psU[:, m, :], lhsT_of(a_sb, k, m),
                                         h16_0[:, k, :],
                                         start=(m == 0 and k == 0),
                                         stop=(m == 1 and k == KT - 1),
                                         skip_group_check=True)

                h16 = h16_0
                for t in range(T):
                    dt = float(dts[t])
                    a1t = a1_sb[:, t, :]
                    wrst = wrs_sb[:, t, :]

                    # ---- ACT 1: k1 = tanh(U); DVE: tts1 = dt*k1 (fp16) ----
                    # GRU tails are evaluated on the Euler prediction
                    # hp ~ h + dt*k1 so the whole gate chain overlaps the
                    # ODE stage-2 chain (costs ~2.7e-3 relmax, gate 2e-2).
                    # The r-tail runs directly on k1h with per-step
                    # dt-prescaled weights (wrs) to skip the tts1 hop.
                    k1h = tmp.tile([128, KT, BL], F16, tag="k1h")
                    nc.scalar.activation(k1h, psU, AF.Tanh)
                    tts1 = tmp.tile([128, KT, BL], F16, tag="tts1")
                    nc.vector.tensor_scalar_mul(tts1, k1h, dt)

                    # ---- PE in readiness order: folds/ps2a/main-r (h16),
                    # then rtail/ps2b (k1h), then main-zn/ztails (tts1) ----
                    psr = ps_r.tile([128, 2, BL], F32, tag="r")
                    psz = ps_z.tile([128, 2, BL], F32, tag="z")
                    psn = ps_n.tile([128, 2, BL], F32, tag="n")
                    nc.tensor.matmul(psr[:, :, :], foldw_sb[0:10, :],
                                     foldx_sb[0:10, t, :],
                                     start=True, stop=False, skip_group_check=True)
                    nc.tensor.matmul(psz[:, :, :], foldw_sb[32:42, :],
                                     foldx_sb[32:42, t, :],
                                     start=True, stop=False, skip_group_check=True)
                    nc.tensor.matmul(psn[:, :, :], foldw_sb[64:68, :],
                                     foldx_sb[64:68, t, :],
                                     start=True, stop=False, skip_group_check=True)
                    ps2 = ps_2.tile([128, 2, BL], F32, tag="ps2")
                    for m in range(2):
                        for k in range(KT):
                            nc.tensor.matmul(ps2[:, m, :], lhsT_of(a_sb, k, m),
                                             h16[:, k, :],
                                             start=(m == 0 and k == 0), stop=False,
                                             skip_group_check=True)
                    psg = [psr, psz, psn]
                    for g in (0, 1):                     # r-gate main
                        for k in range(KT):
                            nc.tensor.matmul(psr[:, g, :], whh_lhsT(k, g),
                                             h16[:, k, :], start=False, stop=False,
                                             skip_group_check=True)
                    for g in range(2):                   # r-tail on k1h (chain)
                        for k in range(KT):
                            nc.tensor.matmul(psr[:, g, :], lhsT_of(wrst, k, g),
                                             k1h[:, k, :], start=False,
                                             stop=(g == 1 and k == KT - 1),
                                             skip_group_check=True)
                    for m in range(2):                   # ps2 += (dt/2 A).T k1
                        for k in range(KT):
                            nc.tensor.matmul(ps2[:, m, :], lhsT_of(a1t, k, m),
                                             k1h[:, k, :], start=False,
                                             stop=(m == 1 and k == KT - 1),
                                             skip_group_check=True)

                    # ---- ACT 2: r = sig(ps_r) ----
                    r = tmp.tile([128, KT, BL], F32, tag="r")
                    nc.scalar.activation(r, psr, AF.Sigmoid)

                    for g in (2, 3, 4, 5):               # z/n main
                        for k in range(KT):
                            nc.tensor.matmul(psg[g // 2][:, g % 2, :],
                                             whh_lhsT(k, g), h16[:, k, :],
                                             start=False, stop=False,
                                             skip_group_check=True)
                    for g in (2, 3, 4, 5):               # z/n tails on tts1
                        for k in range(KT):
                            nc.tensor.matmul(psg[g // 2][:, g % 2, :],
                                             whh_lhsT(k, g), tts1[:, k, :],
                                             start=False,
                                             stop=(g in (3, 5) and k == KT - 1),
                                             skip_group_check=True)

                    # ---- ACT 3,4: 1-z = sig(ps_z); k2 = tanh(ps2) ----
                    zc = tmp.tile([128, KT, BL], F32, tag="zc")
                    nc.scalar.activation(zc, psz, AF.Sigmoid)
                    k2h = tmp.tile([128, KT, BL], F16, tag="k2h")
                    nc.scalar.activation(k2h, ps2, AF.Tanh)

                    # ---- Pool: w = mask * (1-z); wdd16 for the U-update ----
                    m_slice = m_sb[:, t * BL:(t + 1) * BL]
                    m_ap = bass.AP(tensor=m_slice.tensor, offset=m_slice.offset,
                                   ap=[list(m_slice.ap[0]), [0, KT], [1, BL]])
                    w = tmp.tile([128, KT, BL], F32, tag="w")
                    nc.gpsimd.tensor_mul(w, zc, m_ap)

                    # ---- DVE chain: argn = psn*r + gi; ACT 5: n = tanh ----
                    tmpn = tmp.tile([128, KT, BL], F32, tag="tmpn")
                    nc.vector.tensor_mul(tmpn, psn, r)
                    argn = tmp.tile([128, KT, BL], F32, tag="argn")
                    nc.vector.tensor_add(argn, tmpn, gi_sb[:, t, :, :])
                    n = tmp.tile([128, KT, BL], F32, tag="n")
                    nc.scalar.activation(n, argn, AF.Tanh)

                    # ---- fp16 products for the U-update; w*hp split as
                    # w*h + w*dd so the w-branch never waits on k2 ----
                    dd16 = tmp.tile([128, KT, BL], F16, tag="dd16")
                    nc.vector.tensor_scalar_mul(dd16, k2h, dt)
                    e1 = tmp.tile([128, KT, BL], F16, tag="e1")
                    nc.vector.tensor_add(e1, h16, dd16)
                    wh16 = tmp.tile([128, KT, BL], F16, tag="wh16")
                    nc.gpsimd.tensor_mul(wh16, w, h16)
                    e2 = tmp.tile([128, KT, BL], F16, tag="e2")
                    nc.vector.tensor_sub(e2, e1, wh16)
                    wdd16 = tmp.tile([128, KT, BL], F16, tag="wdd16")
                    nc.gpsimd.tensor_mul(wdd16, w, dd16)
                    wn16 = tmp.tile([128, KT, BL], F16, tag="wn16")
                    nc.vector.tensor_mul(wn16, w, n)
                    # early fp16 h for next step's PE work (gh/ps2a)
                    e3 = tmp.tile([128, KT, BL], F16, tag="e3")
                    nc.vector.tensor_sub(e3, e2, wdd16)
                    h16n = state.tile([128, KT, BL], F16, tag="h16")
                    nc.vector.tensor_add(h16n, e3, wn16)

                    # ---- PE: U += A.T dd - A.T wh - A.T wdd + A.T wn ----
                    for src_, lhs in ((dd16, a_sb), (wh16, an_sb),
                                      (wdd16, an_sb)):
                        for m in range(2):
                            for k in range(KT):
                                nc.tensor.matmul(psU[:, m, :], lhsT_of(lhs, k, m),
                                                 src_[:, k, :], start=False,
                                                 stop=False, skip_group_check=True)
                    for m in range(2):
                        for k in range(KT):
                            nc.tensor.matmul(psU[:, m, :], lhsT_of(a_sb, k, m),
                                             wn16[:, k, :], start=False,
                                             stop=(m == 1 and k == KT - 1),
                                             skip_group_check=True)

                    h16 = h16n

                hfin = tmp.tile([128, KT, BL], F32, tag="hfin")
                nc.vector.tensor_copy(hfin, h16)
                return hfin

            if repeat == 1:
                hfin = body()
            else:
                with tc.For_i(0, repeat, 1):
                    hfin = body()

            for k in range(KT):
                nc.sync.dma_start(out=out_d[k, :, :], in_=hfin[:, k, :])

    nc.finalize()
    return nc


def _prepare_inputs(batch, mask, W1, b1, W2, b2, W_ih, b_ih, W_hh, b_hh):
    batch = np.asarray(batch, np.float32)
    mask = np.asarray(mask, np.float32)
    W1 = np.asarray(W1, np.float32); b1 = np.asarray(b1, np.float32)
    W2 = np.asarray(W2, np.float32); b2 = np.asarray(b2, np.float32)
    W_ih = np.asarray(W_ih, np.float32); b_ih = np.asarray(b_ih, np.float32)
    W_hh = np.asarray(W_hh, np.float32); b_hh = np.asarray(b_hh, np.float32)

    A = (W1.T.astype(np.float64) @ W2.T.astype(np.float64)).astype(np.float32)
    c = (b1.astype(np.float64) @ W2.T.astype(np.float64) + b2).astype(np.float32)
    assert np.abs(c).max() == 0.0, "nonzero ODE bias not wired into ACT bias"

    times = batch[0, :, 0].astype(np.float64)
    dts = np.diff(np.concatenate([[0.0], times]))

    def a_blocks(M, dtype=np.float16):   # [H, H] -> [128, KT*H] k-tile concat
        return np.ascontiguousarray(np.concatenate(
            [M[k * 128:(k + 1) * 128, :] for k in range(KT)], axis=1)).astype(dtype)

    a16 = a_blocks(A)
    a16n = a_blocks(-A)
    a1s = np.ascontiguousarray(np.stack(
        [a_blocks((A.astype(np.float64) * (d / 2)).astype(np.float32))
         for d in dts]).transpose(1, 0, 2))              # [128,T,KT*H] fp16
    WhhT_r = W_hh.T[:, 0:H].astype(np.float64)           # r-gate, unnegated
    wrs = np.ascontiguousarray(np.stack(
        [a_blocks((WhhT_r * d).astype(np.float32)) for d in dts]
    ).transpose(1, 0, 2))                                # dt-prescaled r tail
    # z-gate negated so sigmoid(ps_z) = 1 - z directly
    WhhT = np.ascontiguousarray(W_hh.T).copy()
    WhhT[:, H:2 * H] *= -1.0
    whh16 = np.ascontiguousarray(
        np.stack([WhhT[k * 128:(k + 1) * 128, :] for k in range(KT)], axis=1)
    ).astype(np.float16)

    # fold weights: exact fp16 split of W_ih and (b_ih+b_hh) per gate half.
    # lhsT row blocks per region: [Whi, Wlo, Whi, bhi, blo] pairing with rhs
    # rows [xhi, xhi, xlo, 1, 1]; n-gate: [bhi, blo] with ones. All r/z rows
    # live at base partition 0 (rows 0..19) so the single rz fold matmul and
    # the whh accumulates share base partition (mixed-base accumulate after
    # a base-0 start faults on HW). Output slot selection is via zero-padded
    # rhs columns.
    bsum = b_ih + b_hh
    foldw = np.zeros((96, 128), np.float16)
    for reg in range(4):                                 # r0 r1 z0 z1
        sgn = 1.0 if reg < 2 else -1.0                   # z region negated
        wslice = sgn * W_ih[reg * 128:(reg + 1) * 128, 0]
        whi = wslice.astype(np.float16)
        wlo = (wslice - whi.astype(np.float32)).astype(np.float16)
        bs = sgn * bsum[reg * 128:(reg + 1) * 128]
        bshi = bs.astype(np.float16)
        bslo = (bs - bshi.astype(np.float32)).astype(np.float16)
        base = (reg // 2) * 32 + (reg % 2) * 5           # r: 0/5, z: 32/37
        foldw[base + 0] = whi
        foldw[base + 1] = wlo
        foldw[base + 2] = whi
        foldw[base + 3] = bshi
        foldw[base + 4] = bslo
    for reg in range(2):                                 # n0 n1 (b_hh only)
        bn = b_hh[2 * H + reg * 128:2 * H + (reg + 1) * 128]
        bnhi = bn.astype(np.float16)
        bnlo = (bn - bnhi.astype(np.float32)).astype(np.float16)
        foldw[64 + reg * 2 + 0] = bnhi
        foldw[64 + reg * 2 + 1] = bnlo

    xs = batch[:, :, 1]
    gi_n_full = (xs[:, :, None] * W_ih[None, None, 2 * H:, 0]
                 + b_ih[None, None, 2 * H:]).astype(np.float32)  # [B,T,H]

    in_maps = []
    for ci in range(NCORES):
        bs = slice(ci * BL, (ci + 1) * BL)
        xs_c = xs[bs].T                                  # [T, BL]
        xhi = xs_c.astype(np.float16)
        xlo = (xs_c - xhi.astype(np.float32)).astype(np.float16)
        foldx = np.zeros((96, T, 2 * BL), np.float16)
        for reg01, sl in ((0, slice(0, BL)), (1, slice(BL, 2 * BL))):
            for zbase in (0, 32):                        # r rows, z rows (same rhs)
                base = zbase + reg01 * 5
                foldx[base + 0, :, sl] = xhi
                foldx[base + 1, :, sl] = xhi
                foldx[base + 2, :, sl] = xlo
                foldx[base + 3, :, sl] = 1.0
                foldx[base + 4, :, sl] = 1.0
            foldx[64 + reg01 * 2 + 0, :, sl] = 1.0       # n ones
            foldx[64 + reg01 * 2 + 1, :, sl] = 1.0
        mrow = np.ascontiguousarray(mask[bs].T.reshape(1, -1)).astype(np.float32)
        gi_c = gi_n_full[bs].transpose(1, 2, 0)          # [T, H, BL]
        gi_c = np.ascontiguousarray(
            gi_c.reshape(T, KT, 128, BL).transpose(0, 2, 1, 3))
        im = {
            "a16": a16, "a16n": a16n, "whh16": whh16, "a1s": a1s, "wrs": wrs,
            "foldw": foldw, "foldx": np.ascontiguousarray(foldx),
            "mrow": mrow, "gi_n": gi_c,
        }
        in_maps.append(im)
    return dts, in_maps


def kernel(batch, mask, W1, b1, W2, b2, W_ih, b_ih, W_hh, b_hh):
    dts, in_maps = _prepare_inputs(batch, mask, W1, b1, W2, b2,
                                   W_ih, b_ih, W_hh, b_hh)
    nc = _build_program([float(d) for d in dts])
    res = run_bass_kernel_spmd(nc, in_maps, core_ids=list(range(NCORES)))

    out = np.empty((B, H), np.float32)
    for ci in range(NCORES):
        ho = res.results[ci]["h_out"]                    # [KT, 128, BL]
        for k in range(KT):
            out[ci * BL:(ci + 1) * BL, k * 128:(k + 1) * 128] = ho[k].T
    return out
